# revision 24
# baseline (speedup 1.0000x reference)
"""Distributed cosine-attention kernel for TRN2 (8 NeuronCores).

Problem (nn_Attention): B=4, N=2048, D_MODEL=1024, HEADS=16, DIM_HEAD=64
  qkv = x @ w_qkv.T + b_qkv ; q,k l2-normalized over head dim;
  attn = softmax(clip-scale * qn @ kn^T); out = (attn @ v) @ w_out.T

Sharding: core c handles batch b=c//2 and global heads hg*8..hg*8+8 (hg=c%2).
Each core computes a partial out^T [D_MODEL, N]; the host sums the two cores
of each batch and transposes.

Per-core dataflow (no on-device transposes needed anywhere):
  - host passes x[b].T ("xT" [C,T]) and pre-transposed weight shards
  - QK proj -> Q^T/K^T [d-on-partition, tok-free], head pairs packed 64+64
  - V proj -> V [tok-on-partition, d-free]; bias via K=1 matmul
  - l2norm: sum of squares per head via mask matmul; rsqrt via Ln+Exp on
    ScalarE (single activation-table set); broadcast across partitions via
    step-0 DMA; logit scale folded into K^T
  - S^T tiles [keys, queries] via f32r row-packed matmuls (two K=64 heads
    in row groups 0/64); P^T = Exp(S^T - s) over 4-bank PSUM groups (bf16)
  - O^T = P@V and the softmax denominator via bf16 col-packed matmuls
    (tile_position (0,0)/(0,64)), PSUM-accumulated over all key tiles
  - out^T partial = woutT tiles @ O (bf16)
All emission is software-pipelined: chain ops for unit u-1 are emitted after
the bulk matmuls of unit u, so the in-order PE stream never stalls on
ACT/DVE round-trips.
"""
import sys
sys.path.insert(0, "/opt/trn_rl_repo")

from dataclasses import dataclass

import numpy as np

try:
    import ml_dtypes
    ml_bf16 = ml_dtypes.bfloat16
except ImportError:  # pragma: no cover
    ml_bf16 = np.float32

import concourse.bass as bass
import concourse.tile as tile
import concourse.mybir as mybir
from concourse import bacc
from concourse.bass_utils import run_bass_kernel_spmd

F32 = mybir.dt.float32
F32R = mybir.dt.float32r
BF16 = mybir.dt.bfloat16
AF = mybir.ActivationFunctionType

D_MODEL = 1024
HEADS = 16
DIM_HEAD = 64
INNER = HEADS * DIM_HEAD
B = 4
N = 2048
N_CORES = 8
LOG100 = float(np.log(100.0))

_ACT_SET = "natural_log_exp_and_others"
_tables_patched = False


def _patch_act_tables():
    """Make every activation resolve to one table set (it contains ln, exp,
    square, copy, identity) so no ACT_TABLE_LOAD thrash occurs."""
    global _tables_patched
    if _tables_patched:
        return
    orig = bacc.get_activation_tables

    def patched(arch):
        tabs = orig(arch)
        if _ACT_SET in tabs:
            tabs = {k: (v if k == _ACT_SET else set())
                    for k, v in tabs.items()}
        return tabs

    bacc.get_activation_tables = patched
    _tables_patched = True


@dataclass
class Cfg:
    T: int = N
    C: int = D_MODEL
    NH: int = 8
    DH: int = DIM_HEAD
    QB: int = 512
    SG: int = 2             # k-tiles per exp group
    merge_pairs: tuple = (True, True, True, True)

    @property
    def PAIRS(self):
        return self.NH // 2

    @property
    def CT(self):
        return self.C // 128

    @property
    def KT(self):
        return self.T // 128

    @property
    def NQB(self):
        return self.T // self.QB

    @property
    def VW(self):
        return self.NH * self.DH


def build(cfg: Cfg):
    _patch_act_tables()
    T, C, QB = cfg.T, cfg.C, cfg.QB
    PAIRS, CT, KT, NQB, VW = cfg.PAIRS, cfg.CT, cfg.KT, cfg.NQB, cfg.VW
    SG = cfg.SG

    nc = bacc.Bacc("TRN2", target_bir_lowering=False, debug=False,
                   enable_asserts=False)

    xT_d = nc.declare_dram_parameter("xT", [C, T], F32R, isOutput=False)
    wqkT_d = nc.declare_dram_parameter("wqkT", [C, 2 * PAIRS * 128], F32R,
                                       isOutput=False)
    bqk_d = nc.declare_dram_parameter("bqk", [2 * PAIRS, 128, 1], F32,
                                      isOutput=False)
    wvT_d = nc.declare_dram_parameter("wvT", [C, VW], F32R, isOutput=False)
    bv_d = nc.declare_dram_parameter("bv", [1, VW], F32R, isOutput=False)
    woT_d = nc.declare_dram_parameter("woT", [VW, C], BF16, isOutput=False)
    # per-head scale constants: [:, 0]=-s_h (exp bias), [:, 1]=ln(s_h)
    scl_d = nc.declare_dram_parameter("scl", [cfg.NH, 2, 1], F32,
                                      isOutput=False)
    sel2T_d = nc.declare_dram_parameter("sel2T", [128, 2], F32,
                                        isOutput=False)
    out_d = nc.declare_dram_parameter("out", [C, T], F32, isOutput=True)

    with tile.TileContext(nc) as tc:
        with (
            tc.tile_pool(name="const", bufs=1) as const,
            tc.tile_pool(name="dram", bufs=1, space="DRAM") as dram,
            tc.tile_pool(name="xt", bufs=1) as xt_pool,
            tc.tile_pool(name="wcol", bufs=2) as wcol_pool,
            tc.tile_pool(name="qksb", bufs=2) as qk_sb,
            tc.tile_pool(name="norm", bufs=2) as norm_sb,
            tc.tile_pool(name="vtmp", bufs=2) as vtmp_pool,
            tc.tile_pool(name="att", bufs=2) as att_sb,
            tc.tile_pool(name="pt", bufs=2) as pt_pool,
            tc.tile_pool(name="ofin", bufs=1) as ofin_pool,
            tc.tile_pool(name="otout", bufs=2) as ot_pool,
            # PSUM budget (8 banks): mm 2 + sg 4 + pv 1 + lb 1
            tc.tile_pool(name="psmm", bufs=2, space="PSUM") as ps_mm,
            tc.tile_pool(name="pssg", bufs=1, space="PSUM") as ps_sg,
            tc.tile_pool(name="pspv", bufs=1, space="PSUM") as ps_pv,
        ):
            # ---- DRAM spill tensors ----
            qhat_sp = [dram.tile([128, T], F32R, tag=f"qsp{p}",
                                 name=f"qsp{p}") for p in range(PAIRS)]
            khat_sp = [dram.tile([128, T], F32R, tag=f"ksp{p}",
                                 name=f"ksp{p}") for p in range(PAIRS)]
            vhat_sp = dram.tile([KT, 128, VW], BF16, tag="vsp")
            rq_sp = [dram.tile([2, QB], F32, tag=f"rqsp{i}", name=f"rqsp{i}")
                     for i in range(2)]
            rl_sp = [dram.tile([2, QB], F32, tag=f"rlsp{i}", name=f"rlsp{i}")
                     for i in range(2)]

            # ---- constants ----
            scratch_f = const.tile([128, 128], F32, tag="scratch")
            nc.vector.memset(scratch_f, 1.0)
            ones_r = const.tile([1, 128], F32R, tag="ones_r")
            nc.vector.tensor_copy(ones_r, scratch_f[0:1, :])
            sel2T_f = const.tile([128, 2], F32, tag="sel2Tf")
            nc.sync.dma_start(out=sel2T_f, in_=sel2T_d.ap())
            sel2T = const.tile([128, 2], F32R, tag="sel2T")
            nc.vector.tensor_copy(sel2T, sel2T_f)

            nbias_cols = []
            for h in range(cfg.NH):
                col = const.tile([128, 1], F32, tag=f"nb{h}", name=f"nb{h}")
                nc.sync.dma_start(
                    out=col, in_=scl_d.ap()[h, 0:1, :].to_broadcast((128, 1)))
                nbias_cols.append(col)
            lns_cols = []
            for p in range(PAIRS):
                col = const.tile([2, 1], F32, tag=f"lns{p}", name=f"lns{p}")
                nc.sync.dma_start(out=col,
                                  in_=scl_d.ap()[2 * p:2 * p + 2, 1, :])
                lns_cols.append(col)
            zero_col = const.tile([2, 1], F32, tag="zeroc")
            nc.vector.memset(zero_col, 0.0)

            bqk_cols = []
            for it in range(2 * PAIRS):
                col = const.tile([128, 1], F32, tag=f"bqk{it}",
                                 name=f"bqk{it}")
                nc.sync.dma_start(out=col, in_=bqk_d.ap()[it])
                bqk_cols.append(col)
            bv_r = const.tile([1, VW], F32R, tag="bv")
            nc.sync.dma_start(out=bv_r, in_=bv_d.ap())

            wv_res = const.tile([128, CT, VW], F32R, tag="wv_res")
            nc.sync.dma_start(
                out=wv_res,
                in_=wvT_d.ap().rearrange("(ct p) v -> p ct v", p=128))
            wo_res = const.tile([128, PAIRS, C], BF16, tag="wo_res")
            nc.sync.dma_start(
                out=wo_res,
                in_=woT_d.ap().rearrange("(pt p) c -> p pt c", p=128))

            xt = []
            for ct in range(CT):
                t = xt_pool.tile([128, T], F32R, tag=f"xt{ct}",
                                 name=f"xt{ct}")
                nc.sync.dma_start(out=t,
                                  in_=xT_d.ap()[ct * 128:(ct + 1) * 128, :])
                xt.append(t)

            # ================= V projection (pipelined evac) ==============
            pend_v = None

            def flush_v():
                nonlocal pend_v
                if pend_v is None:
                    return
                tt, vps = pend_v
                vtmp = vtmp_pool.tile([128, VW], BF16, tag="vtmp",
                                      name=f"vtmp{tt}")
                nc.vector.tensor_copy(vtmp, vps)
                nc.sync.dma_start(out=vhat_sp[tt], in_=vtmp)
                pend_v = None

            for tt in range(KT):
                vps = ps_mm.tile([128, VW], F32, tag="mm", name=f"vps{tt}")
                for ct in range(CT):
                    nc.tensor.matmul(vps, xt[ct][:, tt * 128:(tt + 1) * 128],
                                     wv_res[:, ct, :], start=(ct == 0),
                                     stop=False)
                nc.tensor.matmul(vps, ones_r[:], bv_r[:], start=False,
                                 stop=True)
                flush_v()
                pend_v = (tt, vps)
            flush_v()

            # ============ QK projection + l2norm (pipelined) ============
            pend_qk = None

            def flush_qk():
                nonlocal pend_qk
                if pend_qk is None:
                    return
                p, is_k, tb, it, qs = pend_qk
                ts = slice(tb * QB, (tb + 1) * QB)
                uid = f"{it}_{tb}"
                qraw = qk_sb.tile([128, QB], F32, tag="qraw",
                                  name=f"qraw{uid}")
                nc.vector.tensor_scalar_add(qraw, qs, bqk_cols[it])
                q2 = qk_sb.tile([128, QB], F32R, tag="q2", name=f"q2{uid}")
                nc.vector.tensor_mul(q2, qraw, qraw)
                ss = ps_mm.tile([2, QB], F32, tag="mm", name=f"ss{uid}")
                nc.tensor.matmul(ss, sel2T[:], q2[:], start=True, stop=True)
                lss = norm_sb.tile([2, QB], F32, tag="lss", name=f"lss{uid}")
                nc.scalar.activation(lss, ss, AF.Ln)
                rq = norm_sb.tile([2, QB], F32, tag="rq", name=f"rq{uid}")
                nc.scalar.activation(rq, lss, AF.Exp, scale=-0.5,
                                     bias=lns_cols[p] if is_k
                                     else zero_col[:])
                rqd = rq_sp[(2 * tb + it) % 2]
                nc.sync.dma_start(out=rqd, in_=rq)
                rqbc = qk_sb.tile([128, QB], F32, tag="rqbc",
                                  name=f"rqbc{uid}")
                nc.sync.dma_start(out=rqbc[0:64, :],
                                  in_=rqd[0:1, :].to_broadcast((64, QB)))
                nc.sync.dma_start(out=rqbc[64:128, :],
                                  in_=rqd[1:2, :].to_broadcast((64, QB)))
                qhat = qk_sb.tile([128, QB], F32R, tag="qhat",
                                  name=f"qhat{uid}")
                nc.vector.tensor_mul(qhat, qraw, rqbc)
                dst = khat_sp[p] if is_k else qhat_sp[p]
                nc.sync.dma_start(out=dst[:, ts], in_=qhat)
                pend_qk = None

            for p in range(PAIRS):
                for is_k in (0, 1):
                    it = 2 * p + is_k
                    wcol = wcol_pool.tile([128, CT, 128], F32R, tag="wcol",
                                          name=f"wcol{it}")
                    nc.sync.dma_start(
                        out=wcol,
                        in_=wqkT_d.ap().rearrange(
                            "(ct pp) i -> pp ct i", pp=128)[
                                :, :, it * 128:(it + 1) * 128])
                    for tb in range(NQB):
                        ts = slice(tb * QB, (tb + 1) * QB)
                        qs = ps_mm.tile([128, QB], F32, tag="mm",
                                        name=f"qs{it}_{tb}")
                        for ct in range(CT):
                            nc.tensor.matmul(qs, wcol[:, ct, :], xt[ct][:, ts],
                                             start=(ct == 0),
                                             stop=(ct == CT - 1))
                        flush_qk()
                        pend_qk = (p, is_k, tb, it, qs)
            flush_qk()

            # ================= attention (pipelined) =================
            o_fin = {}
            NSG = KT // SG
            for p in range(PAIRS):
                kk = att_sb.tile([128, T], F32R, tag="kk", name=f"kk{p}")
                nc.sync.dma_start(out=kk, in_=khat_sp[p])
                qq = att_sb.tile([128, T], F32R, tag="qq", name=f"qq{p}")
                nc.sync.dma_start(out=qq, in_=qhat_sp[p])
                vv = att_sb.tile([128, KT, 2, 65], BF16, tag="vv",
                                 name=f"vv{p}")
                nc.vector.memset(vv[:].rearrange("p a b c -> p (a b c)"), 1.0)
                for j in (0, 1):
                    nc.sync.dma_start(
                        out=vv[:, :, j, 0:64],
                        in_=vhat_sp[:, :, p * 128 + j * 64:
                                    p * 128 + j * 64 + 64].rearrange(
                            "kt pp w -> pp kt w"))
                for qb in range(NQB):
                    qsl = slice(qb * QB, (qb + 1) * QB)
                    pva = ps_pv.tile([65, QB], F32, tag="pv",
                                     name=f"pva{p}_{qb}")
                    pvb = ps_pv.tile([65, QB], F32, tag="lb",
                                     name=f"pvb{p}_{qb}")

                    def emit_pvlb(g, ptile, pva=pva, pvb=pvb, vv=vv):
                        for j in range(SG):
                            kt = g * SG + j
                            first = kt == 0
                            last = kt == KT - 1
                            nc.tensor.matmul(pva, vv[:, kt, 0, :],
                                             ptile[:, 0, j, :], start=first,
                                             stop=last)
                            nc.tensor.matmul(pvb, vv[:, kt, 1, :],
                                             ptile[:, 1, j, :], start=first,
                                             stop=last)

                    pend_att = None
                    for g in range(NSG):
                        sg = ps_sg.tile([128, 2, SG, QB], F32, tag="sg",
                                        name=f"sg{p}_{qb}_{g}")
                        for j in range(SG):
                            kt = g * SG + j
                            ksl = slice(kt * 128, (kt + 1) * 128)
                            nc.tensor.matmul(sg[:, 0, j, :], kk[0:64, ksl],
                                             qq[0:64, qsl], start=True,
                                             stop=True)
                            nc.tensor.matmul(sg[:, 1, j, :], kk[64:128, ksl],
                                             qq[64:128, qsl], start=True,
                                             stop=True)
                        ptile = pt_pool.tile([128, 2, SG, QB], BF16, tag="pt",
                                             name=f"pt{p}_{qb}_{g}")
                        if cfg.merge_pairs[p]:
                            nc.scalar.activation(ptile, sg, AF.Exp,
                                                 bias=nbias_cols[2 * p][:])
                        else:
                            nc.scalar.activation(ptile[:, 0], sg[:, 0],
                                                 AF.Exp,
                                                 bias=nbias_cols[2 * p][:])
                            nc.scalar.activation(
                                ptile[:, 1], sg[:, 1], AF.Exp,
                                bias=nbias_cols[2 * p + 1][:])
                        if pend_att is not None:
                            emit_pvlb(*pend_att)
                        pend_att = (g, ptile)
                    emit_pvlb(*pend_att)

                    rla = att_sb.tile([1, QB], F32, tag="rla",
                                      name=f"rla{p}_{qb}", bufs=1)
                    nc.vector.tensor_copy(rla, pva[64:65, :])
                    rlb = att_sb.tile([1, QB], F32, tag="rlb",
                                      name=f"rlb{p}_{qb}", bufs=1)
                    nc.vector.tensor_copy(rlb, pvb[64:65, :])
                    rld = rl_sp[(p * NQB + qb) % 2]
                    nc.sync.dma_start(out=rld[0:1, :], in_=rla)
                    nc.sync.dma_start(out=rld[1:2, :], in_=rlb)
                    lbc = att_sb.tile([128, QB], F32, tag="lbc",
                                      name=f"lbc{p}_{qb}")
                    nc.sync.dma_start(out=lbc[0:64, :],
                                      in_=rld[0:1, :].to_broadcast((64, QB)))
                    nc.sync.dma_start(out=lbc[64:128, :],
                                      in_=rld[1:2, :].to_broadcast((64, QB)))
                    nc.vector.reciprocal_approx_fast(out=lbc, in_=lbc)
                    of = ofin_pool.tile([128, QB], BF16, tag=f"of{p}_{qb}",
                                        name=f"of{p}_{qb}")
                    nc.vector.tensor_mul(of[0:64, :], pva[0:64, :],
                                         lbc[0:64, :])
                    nc.vector.tensor_mul(of[64:128, :], pvb[0:64, :],
                                         lbc[64:128, :])
                    o_fin[(p, qb)] = of

            # ================= out projection (pipelined evac) ============
            pend_o = None

            def flush_o():
                nonlocal pend_o
                if pend_o is None:
                    return
                qb, cb, op = pend_o
                csl = slice(cb * 128, (cb + 1) * 128)
                ot = ot_pool.tile([128, QB], F32, tag="ot",
                                  name=f"ot{qb}_{cb}")
                nc.vector.tensor_copy(ot, op)
                nc.sync.dma_start(
                    out=out_d.ap()[csl, qb * QB:(qb + 1) * QB], in_=ot)
                pend_o = None

            for qb in range(NQB):
                for cb in range(CT):
                    op = ps_mm.tile([128, QB], F32, tag="mm",
                                    name=f"op{qb}_{cb}")
                    for p in range(PAIRS):
                        nc.tensor.matmul(op, wo_res[:, p,
                                                    cb * 128:(cb + 1) * 128],
                                         o_fin[(p, qb)][:],
                                         start=(p == 0), stop=(p == PAIRS - 1))
                    flush_o()
                    pend_o = (qb, cb, op)
            flush_o()

    nc.compile()
    return nc


# ======================= host-side sharding =======================

def shard_inputs(x, w_qkv, b_qkv, w_out, logit_scale):
    x = np.ascontiguousarray(np.asarray(x, dtype=np.float32))
    w_qkv = np.asarray(w_qkv, dtype=np.float32)
    b_qkv = np.asarray(b_qkv, dtype=np.float32)
    w_out = np.asarray(w_out, dtype=np.float32)
    ls = np.asarray(logit_scale, dtype=np.float32).reshape(-1)
    s_all = np.exp(np.minimum(ls, LOG100)).astype(np.float32)

    Wq = w_qkv[0 * INNER:1 * INNER]
    Wk = w_qkv[1 * INNER:2 * INNER]
    Wv = w_qkv[2 * INNER:3 * INNER]
    bq = b_qkv[0 * INNER:1 * INNER]
    bk = b_qkv[1 * INNER:2 * INNER]
    bv = b_qkv[2 * INNER:3 * INNER]

    xT = [np.ascontiguousarray(x[b].T) for b in range(B)]

    per_hg = {}
    merge = [True] * 4
    for hg in range(2):
        heads = list(range(hg * 8, hg * 8 + 8))
        rows, brows = [], []
        for p in range(4):
            g0, g1 = heads[2 * p], heads[2 * p + 1]
            if s_all[g0] != s_all[g1]:
                merge[p] = False
            rows += [Wq[g0 * 64:(g0 + 1) * 64], Wq[g1 * 64:(g1 + 1) * 64],
                     Wk[g0 * 64:(g0 + 1) * 64], Wk[g1 * 64:(g1 + 1) * 64]]
            brows += [bq[g0 * 64:(g0 + 1) * 64], bq[g1 * 64:(g1 + 1) * 64],
                      bk[g0 * 64:(g0 + 1) * 64], bk[g1 * 64:(g1 + 1) * 64]]
        wqkT = np.ascontiguousarray(np.concatenate(rows, axis=0).T)
        bqk = np.ascontiguousarray(
            np.concatenate(brows, axis=0)).reshape(8, 128, 1)
        vsl = slice(hg * 512, (hg + 1) * 512)
        wvT = np.ascontiguousarray(Wv[vsl].T)
        bvs = np.ascontiguousarray(bv[vsl].reshape(1, 512))
        woT = np.ascontiguousarray(w_out[:, vsl].T.astype(ml_bf16))
        scl = np.stack([-s_all[heads], np.log(s_all[heads])],
                       axis=1).astype(np.float32).reshape(8, 2, 1)
        per_hg[hg] = dict(wqkT=wqkT, bqk=bqk, wvT=wvT, bv=bvs, woT=woT,
                          scl=scl)

    sel2 = np.zeros((2, 128), dtype=np.float32)
    sel2[0, 0:64] = 1.0
    sel2[1, 64:128] = 1.0
    sel2T = np.ascontiguousarray(sel2.T)
    in_maps = []
    for c in range(N_CORES):
        b, hg = c // 2, c % 2
        m = dict(per_hg[hg])
        m["xT"] = xT[b]
        m["sel2T"] = sel2T
        in_maps.append(m)
    return in_maps, tuple(merge)


_NC_CACHE = {}
TRACE = False
LAST_RESULT = None


def kernel(x, w_qkv, b_qkv, w_out, logit_scale):
    global LAST_RESULT
    in_maps, merge_pairs = shard_inputs(x, w_qkv, b_qkv, w_out, logit_scale)
    cfg = Cfg(merge_pairs=merge_pairs)
    if merge_pairs not in _NC_CACHE:
        _NC_CACHE[merge_pairs] = build(cfg)
    nc = _NC_CACHE[merge_pairs]
    res = run_bass_kernel_spmd(nc, in_maps, core_ids=list(range(N_CORES)),
                               trace=TRACE)
    LAST_RESULT = res
    outs = [res.results[c]["out"] for c in range(N_CORES)]
    full = np.empty((B, N, D_MODEL), dtype=np.float32)
    for b in range(B):
        full[b] = (outs[2 * b] + outs[2 * b + 1]).T
    return full


# revision 27
# speedup vs baseline: 1.0058x; 1.0058x over previous
"""Distributed cosine-attention kernel for TRN2 (8 NeuronCores).

Problem (nn_Attention): B=4, N=2048, D_MODEL=1024, HEADS=16, DIM_HEAD=64
  qkv = x @ w_qkv.T + b_qkv ; q,k l2-normalized over head dim;
  attn = softmax(clip-scale * qn @ kn^T); out = (attn @ v) @ w_out.T

Sharding: core c handles batch b=c//2 and global heads hg*8..hg*8+8 (hg=c%2).
Each core computes a partial out^T [D_MODEL, N]; the host sums the two cores
of each batch and transposes.

Per-core dataflow (no on-device transposes needed anywhere):
  - host passes x[b].T ("xT" [C,T]) and pre-transposed weight shards
  - QK proj -> Q^T/K^T [d-on-partition, tok-free], head pairs packed 64+64
  - V proj -> V [tok-on-partition, d-free]; bias via K=1 matmul
  - l2norm: sum of squares per head via mask matmul; rsqrt via Ln+Exp on
    ScalarE (single activation-table set); broadcast across partitions via
    step-0 DMA; logit scale folded into K^T
  - S^T tiles [keys, queries] via f32r row-packed matmuls (two K=64 heads
    in row groups 0/64); P^T = Exp(S^T - s) over 4-bank PSUM groups (bf16)
  - O^T = P@V and the softmax denominator via bf16 col-packed matmuls
    (tile_position (0,0)/(0,64)), PSUM-accumulated over all key tiles
  - out^T partial = woutT tiles @ O (bf16)
All emission is software-pipelined: chain ops for unit u-1 are emitted after
the bulk matmuls of unit u, so the in-order PE stream never stalls on
ACT/DVE round-trips.
"""
import sys
sys.path.insert(0, "/opt/trn_rl_repo")

from dataclasses import dataclass

import numpy as np

try:
    import ml_dtypes
    ml_bf16 = ml_dtypes.bfloat16
except ImportError:  # pragma: no cover
    ml_bf16 = np.float32

import concourse.bass as bass
import concourse.tile as tile
import concourse.mybir as mybir
from concourse import bacc
from concourse.bass_utils import run_bass_kernel_spmd

F32 = mybir.dt.float32
F32R = mybir.dt.float32r
BF16 = mybir.dt.bfloat16
AF = mybir.ActivationFunctionType

D_MODEL = 1024
HEADS = 16
DIM_HEAD = 64
INNER = HEADS * DIM_HEAD
B = 4
N = 2048
N_CORES = 8
LOG100 = float(np.log(100.0))

_ACT_SET = "natural_log_exp_and_others"
_tables_patched = False


def _patch_act_tables():
    """Make every activation resolve to one table set (it contains ln, exp,
    square, copy, identity) so no ACT_TABLE_LOAD thrash occurs."""
    global _tables_patched
    if _tables_patched:
        return
    orig = bacc.get_activation_tables

    def patched(arch):
        tabs = orig(arch)
        if _ACT_SET in tabs:
            tabs = {k: (v if k == _ACT_SET else set())
                    for k, v in tabs.items()}
        return tabs

    bacc.get_activation_tables = patched
    _tables_patched = True


@dataclass
class Cfg:
    T: int = N
    C: int = D_MODEL
    NH: int = 8
    DH: int = DIM_HEAD
    QB: int = 512
    SG: int = 2             # k-tiles per exp group
    merge_pairs: tuple = (True, True, True, True)

    @property
    def PAIRS(self):
        return self.NH // 2

    @property
    def CT(self):
        return self.C // 128

    @property
    def KT(self):
        return self.T // 128

    @property
    def NQB(self):
        return self.T // self.QB

    @property
    def VW(self):
        return self.NH * self.DH


def build(cfg: Cfg):
    _patch_act_tables()
    T, C, QB = cfg.T, cfg.C, cfg.QB
    PAIRS, CT, KT, NQB, VW = cfg.PAIRS, cfg.CT, cfg.KT, cfg.NQB, cfg.VW
    SG = cfg.SG

    nc = bacc.Bacc("TRN2", target_bir_lowering=False, debug=False,
                   enable_asserts=False)

    xT_d = nc.declare_dram_parameter("xT", [C, T], F32R, isOutput=False)
    wqkT_d = nc.declare_dram_parameter("wqkT", [C, 2 * PAIRS * 128], F32R,
                                       isOutput=False)
    bqk_d = nc.declare_dram_parameter("bqk", [2 * PAIRS, 128, 1], F32,
                                      isOutput=False)
    wvT_d = nc.declare_dram_parameter("wvT", [C, VW], F32R, isOutput=False)
    bv_d = nc.declare_dram_parameter("bv", [1, VW], F32R, isOutput=False)
    woT_d = nc.declare_dram_parameter("woT", [VW, C], BF16, isOutput=False)
    # per-head scale constants: [:, 0]=-s_h (exp bias), [:, 1]=ln(s_h)
    scl_d = nc.declare_dram_parameter("scl", [cfg.NH, 2, 1], F32,
                                      isOutput=False)
    sel2T_d = nc.declare_dram_parameter("sel2T", [128, 2], F32,
                                        isOutput=False)
    out_d = nc.declare_dram_parameter("out", [C, T], F32, isOutput=True)

    with tile.TileContext(nc) as tc:
        with (
            tc.tile_pool(name="const", bufs=1) as const,
            tc.tile_pool(name="dram", bufs=1, space="DRAM") as dram,
            tc.tile_pool(name="xt", bufs=1) as xt_pool,
            tc.tile_pool(name="wcol", bufs=2) as wcol_pool,
            tc.tile_pool(name="qksb", bufs=2) as qk_sb,
            tc.tile_pool(name="norm", bufs=2) as norm_sb,
            tc.tile_pool(name="vtmp", bufs=2) as vtmp_pool,
            tc.tile_pool(name="att", bufs=2) as att_sb,
            tc.tile_pool(name="pt", bufs=2) as pt_pool,
            tc.tile_pool(name="ofin", bufs=1) as ofin_pool,
            tc.tile_pool(name="otout", bufs=2) as ot_pool,
            # PSUM budget (8 banks): mm 2 + sg 4 + pv 1 + lb 1
            tc.tile_pool(name="psmm", bufs=2, space="PSUM") as ps_mm,
            tc.tile_pool(name="pssg", bufs=1, space="PSUM") as ps_sg,
            tc.tile_pool(name="pspv", bufs=1, space="PSUM") as ps_pv,
        ):
            # ---- DRAM spill tensors ----
            qhat_sp = [dram.tile([128, T], F32R, tag=f"qsp{p}",
                                 name=f"qsp{p}") for p in range(PAIRS)]
            khat_sp = [dram.tile([128, T], F32R, tag=f"ksp{p}",
                                 name=f"ksp{p}") for p in range(PAIRS)]
            vhat_sp = dram.tile([KT, 128, VW], BF16, tag="vsp")
            rq_sp = [dram.tile([2, QB], F32, tag=f"rqsp{i}", name=f"rqsp{i}")
                     for i in range(2)]
            rl_sp = [dram.tile([2, QB], F32, tag=f"rlsp{i}", name=f"rlsp{i}")
                     for i in range(2)]

            # ---- constants ----
            scratch_f = const.tile([128, 128], F32, tag="scratch")
            nc.vector.memset(scratch_f, 1.0)
            ones_r = const.tile([1, 128], F32R, tag="ones_r")
            nc.vector.tensor_copy(ones_r, scratch_f[0:1, :])
            sel2T_f = const.tile([128, 2], F32, tag="sel2Tf")
            nc.sync.dma_start(out=sel2T_f, in_=sel2T_d.ap())
            sel2T = const.tile([128, 2], F32R, tag="sel2T")
            nc.vector.tensor_copy(sel2T, sel2T_f)

            nbias_cols = []
            for h in range(cfg.NH):
                col = const.tile([128, 1], F32, tag=f"nb{h}", name=f"nb{h}")
                nc.sync.dma_start(
                    out=col, in_=scl_d.ap()[h, 0:1, :].to_broadcast((128, 1)))
                nbias_cols.append(col)
            lns_cols = []
            for p in range(PAIRS):
                col = const.tile([2, 1], F32, tag=f"lns{p}", name=f"lns{p}")
                nc.sync.dma_start(out=col,
                                  in_=scl_d.ap()[2 * p:2 * p + 2, 1, :])
                lns_cols.append(col)
            zero_col = const.tile([2, 1], F32, tag="zeroc")
            nc.vector.memset(zero_col, 0.0)

            bqk_cols = []
            for it in range(2 * PAIRS):
                col = const.tile([128, 1], F32, tag=f"bqk{it}",
                                 name=f"bqk{it}")
                nc.sync.dma_start(out=col, in_=bqk_d.ap()[it])
                bqk_cols.append(col)
            bv_r = const.tile([1, VW], F32R, tag="bv")
            nc.sync.dma_start(out=bv_r, in_=bv_d.ap())

            wv_res = const.tile([128, CT, VW], F32R, tag="wv_res")
            nc.sync.dma_start(
                out=wv_res,
                in_=wvT_d.ap().rearrange("(ct p) v -> p ct v", p=128))
            wo_res = const.tile([128, PAIRS, C], BF16, tag="wo_res")
            nc.sync.dma_start(
                out=wo_res,
                in_=woT_d.ap().rearrange("(pt p) c -> p pt c", p=128))

            xt = []
            for ct in range(CT):
                t = xt_pool.tile([128, T], F32R, tag=f"xt{ct}",
                                 name=f"xt{ct}")
                nc.sync.dma_start(out=t,
                                  in_=xT_d.ap()[ct * 128:(ct + 1) * 128, :])
                xt.append(t)

            # ================= V projection (pipelined evac) ==============
            pend_v = None

            def flush_v():
                nonlocal pend_v
                if pend_v is None:
                    return
                tt, vps = pend_v
                vtmp = vtmp_pool.tile([128, VW], BF16, tag="vtmp",
                                      name=f"vtmp{tt}")
                nc.vector.tensor_copy(vtmp, vps)
                nc.sync.dma_start(out=vhat_sp[tt], in_=vtmp)
                pend_v = None

            for tt in range(KT):
                vps = ps_mm.tile([128, VW], F32, tag="mm", name=f"vps{tt}")
                for ct in range(CT):
                    nc.tensor.matmul(vps, xt[ct][:, tt * 128:(tt + 1) * 128],
                                     wv_res[:, ct, :], start=(ct == 0),
                                     stop=False)
                nc.tensor.matmul(vps, ones_r[:], bv_r[:], start=False,
                                 stop=True)
                flush_v()
                pend_v = (tt, vps)
            flush_v()

            # ============ QK projection + l2norm (pipelined) ============
            pend_qk = None

            def flush_qk():
                nonlocal pend_qk
                if pend_qk is None:
                    return
                p, is_k, tb, it, qs = pend_qk
                ts = slice(tb * QB, (tb + 1) * QB)
                uid = f"{it}_{tb}"
                qraw = qk_sb.tile([128, QB], F32, tag="qraw",
                                  name=f"qraw{uid}")
                nc.vector.tensor_scalar_add(qraw, qs, bqk_cols[it])
                q2 = qk_sb.tile([128, QB], F32R, tag="q2", name=f"q2{uid}")
                nc.vector.tensor_mul(q2, qraw, qraw)
                ss = ps_mm.tile([2, QB], F32, tag="mm", name=f"ss{uid}")
                nc.tensor.matmul(ss, sel2T[:], q2[:], start=True, stop=True)
                lss = norm_sb.tile([2, QB], F32, tag="lss", name=f"lss{uid}")
                nc.scalar.activation(lss, ss, AF.Ln)
                rq = norm_sb.tile([2, QB], F32, tag="rq", name=f"rq{uid}")
                nc.scalar.activation(rq, lss, AF.Exp, scale=-0.5,
                                     bias=lns_cols[p] if is_k
                                     else zero_col[:])
                rqd = rq_sp[(2 * tb + it) % 2]
                nc.sync.dma_start(out=rqd, in_=rq)
                rqbc = qk_sb.tile([128, QB], F32, tag="rqbc",
                                  name=f"rqbc{uid}")
                nc.sync.dma_start(out=rqbc[0:64, :],
                                  in_=rqd[0:1, :].to_broadcast((64, QB)))
                nc.sync.dma_start(out=rqbc[64:128, :],
                                  in_=rqd[1:2, :].to_broadcast((64, QB)))
                qhat = qk_sb.tile([128, QB], F32R, tag="qhat",
                                  name=f"qhat{uid}")
                nc.vector.tensor_mul(qhat, qraw, rqbc)
                dst = khat_sp[p] if is_k else qhat_sp[p]
                nc.sync.dma_start(out=dst[:, ts], in_=qhat)
                pend_qk = None

            def emit_proj_pair(p):
                nonlocal pend_qk
                for is_k in (0, 1):
                    it = 2 * p + is_k
                    wcol = wcol_pool.tile([128, CT, 128], F32R, tag="wcol",
                                          name=f"wcol{it}")
                    nc.sync.dma_start(
                        out=wcol,
                        in_=wqkT_d.ap().rearrange(
                            "(ct pp) i -> pp ct i", pp=128)[
                                :, :, it * 128:(it + 1) * 128])
                    for tb in range(NQB):
                        ts = slice(tb * QB, (tb + 1) * QB)
                        qs = ps_mm.tile([128, QB], F32, tag="mm",
                                        name=f"qs{it}_{tb}")
                        for ct in range(CT):
                            nc.tensor.matmul(qs, wcol[:, ct, :], xt[ct][:, ts],
                                             start=(ct == 0),
                                             stop=(ct == CT - 1))
                        flush_qk()
                        pend_qk = (p, is_k, tb, it, qs)

            # ================= attention (pipelined) =================
            o_fin = {}
            NSG = KT // SG

            def emit_att_pair(p):
                kk = att_sb.tile([128, T], F32R, tag="kk", name=f"kk{p}")
                nc.sync.dma_start(out=kk, in_=khat_sp[p])
                qq = att_sb.tile([128, T], F32R, tag="qq", name=f"qq{p}")
                nc.sync.dma_start(out=qq, in_=qhat_sp[p])
                vv = att_sb.tile([128, KT, 2, 65], BF16, tag="vv",
                                 name=f"vv{p}")
                nc.vector.memset(vv[:].rearrange("p a b c -> p (a b c)"), 1.0)
                for j in (0, 1):
                    nc.sync.dma_start(
                        out=vv[:, :, j, 0:64],
                        in_=vhat_sp[:, :, p * 128 + j * 64:
                                    p * 128 + j * 64 + 64].rearrange(
                            "kt pp w -> pp kt w"))
                for qb in range(NQB):
                    qsl = slice(qb * QB, (qb + 1) * QB)
                    pva = ps_pv.tile([65, QB], F32, tag="pv",
                                     name=f"pva{p}_{qb}")
                    pvb = ps_pv.tile([65, QB], F32, tag="lb",
                                     name=f"pvb{p}_{qb}")

                    def emit_pvlb(g, ptile, pva=pva, pvb=pvb, vv=vv):
                        for j in range(SG):
                            kt = g * SG + j
                            first = kt == 0
                            last = kt == KT - 1
                            nc.tensor.matmul(pva, vv[:, kt, 0, :],
                                             ptile[:, 0, j, :], start=first,
                                             stop=last)
                            nc.tensor.matmul(pvb, vv[:, kt, 1, :],
                                             ptile[:, 1, j, :], start=first,
                                             stop=last)

                    pend_att = None
                    for g in range(NSG):
                        sg = ps_sg.tile([128, 2, SG, QB], F32, tag="sg",
                                        name=f"sg{p}_{qb}_{g}")
                        for j in range(SG):
                            kt = g * SG + j
                            ksl = slice(kt * 128, (kt + 1) * 128)
                            nc.tensor.matmul(sg[:, 0, j, :], kk[0:64, ksl],
                                             qq[0:64, qsl], start=True,
                                             stop=True)
                            nc.tensor.matmul(sg[:, 1, j, :], kk[64:128, ksl],
                                             qq[64:128, qsl], start=True,
                                             stop=True)
                        ptile = pt_pool.tile([128, 2, SG, QB], BF16, tag="pt",
                                             name=f"pt{p}_{qb}_{g}")
                        if cfg.merge_pairs[p]:
                            nc.scalar.activation(ptile, sg, AF.Exp,
                                                 bias=nbias_cols[2 * p][:])
                        else:
                            nc.scalar.activation(ptile[:, 0], sg[:, 0],
                                                 AF.Exp,
                                                 bias=nbias_cols[2 * p][:])
                            nc.scalar.activation(
                                ptile[:, 1], sg[:, 1], AF.Exp,
                                bias=nbias_cols[2 * p + 1][:])
                        if pend_att is not None:
                            emit_pvlb(*pend_att)
                        pend_att = (g, ptile)
                    emit_pvlb(*pend_att)

                    rla = att_sb.tile([1, QB], F32, tag="rla",
                                      name=f"rla{p}_{qb}", bufs=1)
                    nc.vector.tensor_copy(rla, pva[64:65, :])
                    rlb = att_sb.tile([1, QB], F32, tag="rlb",
                                      name=f"rlb{p}_{qb}", bufs=1)
                    nc.vector.tensor_copy(rlb, pvb[64:65, :])
                    rld = rl_sp[(p * NQB + qb) % 2]
                    nc.sync.dma_start(out=rld[0:1, :], in_=rla)
                    nc.sync.dma_start(out=rld[1:2, :], in_=rlb)
                    lbc = att_sb.tile([128, QB], F32, tag="lbc",
                                      name=f"lbc{p}_{qb}")
                    nc.sync.dma_start(out=lbc[0:64, :],
                                      in_=rld[0:1, :].to_broadcast((64, QB)))
                    nc.sync.dma_start(out=lbc[64:128, :],
                                      in_=rld[1:2, :].to_broadcast((64, QB)))
                    nc.vector.reciprocal_approx_fast(out=lbc, in_=lbc)
                    of = ofin_pool.tile([128, QB], BF16, tag=f"of{p}_{qb}",
                                        name=f"of{p}_{qb}")
                    nc.vector.tensor_mul(of[0:64, :], pva[0:64, :],
                                         lbc[0:64, :])
                    nc.vector.tensor_mul(of[64:128, :], pvb[0:64, :],
                                         lbc[64:128, :])
                    o_fin[(p, qb)] = of
                    if p == PAIRS - 1:
                        emit_outproj_qb(qb)

            # ================= out projection (pipelined evac) ============
            pend_o = None

            def flush_o():
                nonlocal pend_o
                if pend_o is None:
                    return
                qb, cb, op = pend_o
                csl = slice(cb * 128, (cb + 1) * 128)
                ot = ot_pool.tile([128, QB], F32, tag="ot",
                                  name=f"ot{qb}_{cb}")
                nc.vector.tensor_copy(ot, op)
                nc.sync.dma_start(
                    out=out_d.ap()[csl, qb * QB:(qb + 1) * QB], in_=ot)
                pend_o = None

            def emit_outproj_qb(qb):
                nonlocal pend_o
                for cb in range(CT):
                    op = ps_mm.tile([128, QB], F32, tag="mm",
                                    name=f"op{qb}_{cb}")
                    for p in range(PAIRS):
                        nc.tensor.matmul(op, wo_res[:, p,
                                                    cb * 128:(cb + 1) * 128],
                                         o_fin[(p, qb)][:],
                                         start=(p == 0), stop=(p == PAIRS - 1))
                    flush_o()
                    pend_o = (qb, cb, op)

            # ======== interleaved pair-level schedule ========
            emit_proj_pair(0)
            for p in range(PAIRS):
                if p + 1 < PAIRS:
                    emit_proj_pair(p + 1)
                flush_qk()
                emit_att_pair(p)
            flush_o()

    nc.compile()
    return nc


# ======================= host-side sharding =======================

def shard_inputs(x, w_qkv, b_qkv, w_out, logit_scale):
    x = np.ascontiguousarray(np.asarray(x, dtype=np.float32))
    w_qkv = np.asarray(w_qkv, dtype=np.float32)
    b_qkv = np.asarray(b_qkv, dtype=np.float32)
    w_out = np.asarray(w_out, dtype=np.float32)
    ls = np.asarray(logit_scale, dtype=np.float32).reshape(-1)
    s_all = np.exp(np.minimum(ls, LOG100)).astype(np.float32)

    Wq = w_qkv[0 * INNER:1 * INNER]
    Wk = w_qkv[1 * INNER:2 * INNER]
    Wv = w_qkv[2 * INNER:3 * INNER]
    bq = b_qkv[0 * INNER:1 * INNER]
    bk = b_qkv[1 * INNER:2 * INNER]
    bv = b_qkv[2 * INNER:3 * INNER]

    xT = [np.ascontiguousarray(x[b].T) for b in range(B)]

    per_hg = {}
    merge = [True] * 4
    for hg in range(2):
        heads = list(range(hg * 8, hg * 8 + 8))
        rows, brows = [], []
        for p in range(4):
            g0, g1 = heads[2 * p], heads[2 * p + 1]
            if s_all[g0] != s_all[g1]:
                merge[p] = False
            rows += [Wq[g0 * 64:(g0 + 1) * 64], Wq[g1 * 64:(g1 + 1) * 64],
                     Wk[g0 * 64:(g0 + 1) * 64], Wk[g1 * 64:(g1 + 1) * 64]]
            brows += [bq[g0 * 64:(g0 + 1) * 64], bq[g1 * 64:(g1 + 1) * 64],
                      bk[g0 * 64:(g0 + 1) * 64], bk[g1 * 64:(g1 + 1) * 64]]
        wqkT = np.ascontiguousarray(np.concatenate(rows, axis=0).T)
        bqk = np.ascontiguousarray(
            np.concatenate(brows, axis=0)).reshape(8, 128, 1)
        vsl = slice(hg * 512, (hg + 1) * 512)
        wvT = np.ascontiguousarray(Wv[vsl].T)
        bvs = np.ascontiguousarray(bv[vsl].reshape(1, 512))
        woT = np.ascontiguousarray(w_out[:, vsl].T.astype(ml_bf16))
        scl = np.stack([-s_all[heads], np.log(s_all[heads])],
                       axis=1).astype(np.float32).reshape(8, 2, 1)
        per_hg[hg] = dict(wqkT=wqkT, bqk=bqk, wvT=wvT, bv=bvs, woT=woT,
                          scl=scl)

    sel2 = np.zeros((2, 128), dtype=np.float32)
    sel2[0, 0:64] = 1.0
    sel2[1, 64:128] = 1.0
    sel2T = np.ascontiguousarray(sel2.T)
    in_maps = []
    for c in range(N_CORES):
        b, hg = c // 2, c % 2
        m = dict(per_hg[hg])
        m["xT"] = xT[b]
        m["sel2T"] = sel2T
        in_maps.append(m)
    return in_maps, tuple(merge)


_NC_CACHE = {}
TRACE = False
LAST_RESULT = None


def kernel(x, w_qkv, b_qkv, w_out, logit_scale):
    global LAST_RESULT
    in_maps, merge_pairs = shard_inputs(x, w_qkv, b_qkv, w_out, logit_scale)
    cfg = Cfg(merge_pairs=merge_pairs)
    if merge_pairs not in _NC_CACHE:
        _NC_CACHE[merge_pairs] = build(cfg)
    nc = _NC_CACHE[merge_pairs]
    res = run_bass_kernel_spmd(nc, in_maps, core_ids=list(range(N_CORES)),
                               trace=TRACE)
    LAST_RESULT = res
    outs = [res.results[c]["out"] for c in range(N_CORES)]
    full = np.empty((B, N, D_MODEL), dtype=np.float32)
    for b in range(B):
        full[b] = (outs[2 * b] + outs[2 * b + 1]).T
    return full
